# revision 1
# baseline (speedup 1.0000x reference)
"""GCN (gather-scale-segment_max x2) on 8 Trainium2 NeuronCores.

Strategy (inspector-executor, 4 SPMD launches):
  Edges are sharded by destination-node block (12500 nodes per core), so each
  core owns the complete reduction for its nodes and no cross-core reduce is
  needed. Per layer, the per-edge gather x[src] runs ON DEVICE as a
  run-length-grouped broadcast-expand DMA (edges sorted by src; x staged in a
  host-permuted row order so every group is one static access pattern). The
  host only re-orders device-produced bytes (index take / pad) between
  launches; all value-level gather, arithmetic, reductions and matmuls happen
  on the NeuronCores.
    launch 1: expand x rows per edge (src order)        [DMA only]
    launch 2: msg*ts, two-level segment max, @W1+b1, relu
    launch 3: expand h rows per edge (same structure)
    launch 4: msg*ts, segment max, @W2+b2
"""

import os

os.environ.setdefault("JAX_COMPILATION_CACHE_DIR", "/tmp/jax_kernel_cache")

import numpy as np
from concourse import bass, mybir
from concourse.bass_utils import run_bass_kernel_spmd
from concourse.tile import TileContext
from bass_rust import ScopedClock

try:
    import jax
    jax.config.update("jax_compilation_cache_dir", "/tmp/jax_kernel_cache")
    jax.config.update("jax_persistent_cache_min_compile_time_secs", 0.5)
except Exception:
    pass

N_CORES = 8
N_NODES = 100000
B = N_NODES // N_CORES          # 12500 dst nodes per core
P = 128
F1, HID, NCLS = 16, 8, 2
LMAX = 32                        # max expand run length
EXP_CH = 1024                    # rows per expand DMA instruction
STRIPE_ROWS = 32                 # 32 rows x 16 slots = 512 slots per stripe

_DT = mybir.dt.float32


# ---------------------------------------------------------------- tile patch
class _Tc(TileContext):
    """This walrus build allows only ONE sync-wait per instruction; split the
    end-of-kernel drain waits across SP nops."""

    def _drain_and_barrier(self, tick_clock, wait_clock):
        holder = self.nc.sync.nop(nofuse=True, hint="drain_waits")
        wait_clock.add_sem_waits(holder.ins, ScopedClock({None: tick_clock.global_clock}))
        si = holder.ins.sync_info
        waits = list(si.on_wait) if si and si.on_wait else []
        if len(waits) > 1:
            upd = list(si.on_update) if si.on_update else []
            holder.ins.sync_info = mybir.SyncInfo(on_wait=waits[:1], on_update=upd)
            for w in waits[1:]:
                extra = self.nc.sync.nop(nofuse=True, hint="drain_waits")
                extra.ins.sync_info = mybir.SyncInfo(on_wait=[w], on_update=[])
        self.nc.sync.drain()
        self.nc.all_engine_barrier()
        assert self.sems is not None
        popped = self.nc._tile_sem_poison_stack.pop()
        assert popped is self._sem_poison
        self.nc.clear_and_free_semaphores(list(self.sems.allocated().values()))
        self.nc.all_engine_barrier()


def _split_waits(nc, max_waits=1):
    n = 0
    for fn in nc.m.functions:
        for bb in fn.blocks:
            out = []
            for inst in bb.instructions:
                si = inst.sync_info
                waits = list(si.on_wait) if si and si.on_wait else []
                if len(waits) > max_waits:
                    for w in waits[:-max_waits]:
                        n += 1
                        nop = mybir.InstNoOp(name=f"I-ws-{n}")
                        nop.engine = inst.engine
                        nop.sync_info = mybir.SyncInfo(on_wait=[w], on_update=[])
                        out.append(nop)
                    inst.sync_info = mybir.SyncInfo(
                        on_wait=waits[-max_waits:],
                        on_update=list(si.on_update) if si.on_update else [],
                    )
                out.append(inst)
            bb.instructions[:] = out
    return n


# ------------------------------------------------------------- host indexing
def _ragged_arange(lens):
    """concat([arange(l) for l in lens]) vectorized."""
    lens = np.asarray(lens, dtype=np.int64)
    total = int(lens.sum())
    cum = np.cumsum(lens) - lens
    return np.arange(total, dtype=np.int64) - np.repeat(cum, lens)


def _phase1_rows(es):
    """Rows of the src-sorted expand: (order_s, row_node, row_len, row_start)."""
    order_s = np.argsort(es, kind="stable").astype(np.int64)
    ss = es[order_s]
    u, ufirst, cnt = np.unique(ss, return_index=True, return_counts=True)
    nrows = -(-cnt // LMAX)
    row_node = np.repeat(u, nrows).astype(np.int64)
    total_rows = int(nrows.sum())
    row_len = np.full(total_rows, LMAX, np.int64)
    last = np.cumsum(nrows) - 1
    row_len[last] = cnt - (nrows - 1) * LMAX
    within = _ragged_arange(nrows)
    row_start = np.repeat(ufirst, nrows).astype(np.int64) + within * LMAX
    return order_s, row_node, row_len, row_start


class _Shard:
    pass


def _prepare(src, dst, ts):
    """Per-core shards plus cross-core-uniform structures."""
    shards = []
    blk = dst // B
    for n in range(N_CORES):
        sh = _Shard()
        sel = np.nonzero(blk == n)[0]
        sh.es = src[sel].astype(np.int64)
        sh.ed = (dst[sel] - n * B).astype(np.int64)
        sh.ts = ts[sel]
        sh.En = len(sel)
        (sh.order_s, sh.row_node, sh.row_len, sh.row_start) = _phase1_rows(sh.es)
        shards.append(sh)

    # --- uniform expand-group structure (R_common[l] rows of each length l)
    R_common = np.zeros(LMAX + 1, np.int64)
    for sh in shards:
        R_common = np.maximum(R_common, np.bincount(sh.row_len, minlength=LMAX + 1))
    # round row counts up to multiples of 128 so the expand can run through
    # 128-partition SBUF tiles (one row per partition -> 16-queue DMA fanout)
    lens_used = np.nonzero(R_common)[0]
    R_common = (-(-R_common // P) * P)
    group_base_row = {}
    group_base_pos = {}
    rtot = 0
    etot = 0
    for l in lens_used:
        group_base_row[l] = rtot
        group_base_pos[l] = etot
        rtot += int(R_common[l])
        etot += int(R_common[l]) * int(l)
    RTOT, ETOT = rtot, etot

    for sh in shards:
        # x_perm row list + per-edge expand positions
        xrows = np.zeros(RTOT, np.int64)
        pos_of_edge = np.empty(sh.En, np.int64)
        for l in lens_used:
            rows_l = np.nonzero(sh.row_len == l)[0]
            k = len(rows_l)
            base_r, base_p = group_base_row[l], group_base_pos[l]
            if k:
                xrows[base_r:base_r + k] = sh.row_node[rows_l]
                starts = sh.row_start[rows_l]
                li = int(l)
                idx_in_order = (starts[:, None] + np.arange(li)[None, :]).ravel()
                pos = (base_p + np.arange(k)[:, None] * li + np.arange(li)[None, :]).ravel()
                pos_of_edge[sh.order_s[idx_in_order]] = pos
        sh.xrows = xrows
        sh.pos_of_edge = pos_of_edge

        # --- dst side: degrees, row counts
        sh.order_d = np.argsort(sh.ed, kind="stable").astype(np.int64)
        sh.deg = np.bincount(sh.ed, minlength=B).astype(np.int64)
        assert sh.deg.min() >= 1
        sh.dstart = np.concatenate([[0], np.cumsum(sh.deg)[:-1]])
        sh.rows_i = -(-sh.deg // 16)

    # --- uniform rowcount-group structure
    rmax = max(int(sh.rows_i.max()) for sh in shards)
    count_r = np.zeros(rmax + 1, np.int64)
    for sh in shards:
        count_r = np.maximum(count_r, np.bincount(sh.rows_i, minlength=rmax + 1))
    count_r[0] = 0
    m_r = -(-count_r // P)          # node-grid rows per rowcount group
    rows_pp = int((m_r * np.arange(rmax + 1)).sum())
    pad_rows = (-rows_pp) % STRIPE_ROWS
    if pad_rows:
        m_r[1] += pad_rows          # dummy single-row nodes to align stripes
        rows_pp += pad_rows
    ROWS_PP = rows_pp
    S_PP = ROWS_PP * 16
    M = int(m_r.sum())
    r_list = [int(r) for r in np.nonzero(m_r)[0]]
    groups = [(r, int(m_r[r])) for r in r_list]

    for sh in shards:
        grids = []
        slot_chunks = []
        for r in r_list:
            nodes = np.nonzero(sh.rows_i == r)[0]
            need = int(m_r[r]) * P
            g = np.full(need, -1, np.int64)
            g[:len(nodes)] = nodes
            g = g.reshape(int(m_r[r]), P)
            grids.append(g)
            gg = np.where(g < 0, 0, g)
            start = sh.dstart[gg][:, :, None]
            degg = sh.deg[gg][:, :, None]
            j = np.arange(16 * r)[None, None, :]
            eidx = sh.order_d[start + np.minimum(j, degg - 1)]
            slot_chunks.append(eidx.transpose(1, 0, 2).reshape(P, -1))
        sh.node_grid = np.concatenate(grids, axis=0)          # [M, P]
        sh.slot_edge = np.concatenate(slot_chunks, axis=1)    # [P, S_PP]
        assert sh.slot_edge.shape == (P, S_PP)

    cfg = dict(lens_used=[int(l) for l in lens_used],
               R_common={int(l): int(R_common[l]) for l in lens_used},
               group_base_row={int(k): v for k, v in group_base_row.items()},
               group_base_pos={int(k): v for k, v in group_base_pos.items()},
               RTOT=RTOT, ETOT=ETOT, GROUPS=groups, ROWS_PP=ROWS_PP,
               S_PP=S_PP, M=M)
    return shards, cfg


# ------------------------------------------------------------ device builds
def _build_expand(cfg, feat):
    """Broadcast expand xp[row] -> row_len replicas, routed through SBUF
    128-partition tiles (one source row per partition) so both DMA legs fan
    out across the 16 DMA queues; Tile double-buffers the two HWDGE engines."""
    nc = bass.Bass("TRN2", target_bir_lowering=False, debug=False,
                   num_devices=N_CORES)
    xp = nc.declare_dram_parameter("xp", [cfg["RTOT"], feat], _DT, isOutput=False)
    msgs = nc.declare_dram_parameter("msgs", [cfg["ETOT"], feat], _DT, isOutput=True)
    with _Tc(nc) as tc:
        with tc.tile_pool(name="t", bufs=8) as pool:
            for l in cfg["lens_used"]:
                R = cfg["R_common"][l]
                base_r = cfg["group_base_row"][l]
                base_p = cfg["group_base_pos"][l]
                for c in range(R // P):
                    tl = pool.tile([P, l, feat], _DT, tag="tl")
                    src = xp[base_r + P * c: base_r + P * (c + 1), None, :] \
                        .to_broadcast([P, l, feat])
                    nc.sync.dma_start(out=tl[:, :, :], in_=src)
                    dst = msgs[base_p + P * c * l: base_p + P * (c + 1) * l, :] \
                        .rearrange("(p l) f -> p l f", l=l)
                    nc.scalar.dma_start(out=dst, in_=tl[:, :, :])
    _split_waits(nc)
    return nc


def _build_reduce(cfg, feat, hidden, relu):
    """msg*ts -> two-level segmented max -> (@W + b) [-> relu] -> hT."""
    S_PP, ROWS_PP, M = cfg["S_PP"], cfg["ROWS_PP"], cfg["M"]
    n_stripes = ROWS_PP // STRIPE_ROWS
    SS = STRIPE_ROWS * 16

    nc = bass.Bass("TRN2", target_bir_lowering=False, debug=False,
                   num_devices=N_CORES)
    msg = nc.declare_dram_parameter("msg", [P, S_PP, feat], _DT, isOutput=False)
    tsd = nc.declare_dram_parameter("ts", [P, S_PP], _DT, isOutput=False)
    wd = nc.declare_dram_parameter("w", [feat, hidden], _DT, isOutput=False)
    bd = nc.declare_dram_parameter("b", [hidden, 1], _DT, isOutput=False)
    ident = nc.declare_dram_parameter("ident", [P, P], _DT, isOutput=False)
    hT = nc.declare_dram_parameter("hT", [hidden, M, P], _DT, isOutput=True)

    with _Tc(nc) as tc:
        with tc.tile_pool(name="sb", bufs=3) as sb, \
             tc.tile_pool(name="big", bufs=1) as big, \
             tc.tile_pool(name="cst", bufs=1) as cst, \
             tc.tile_pool(name="ps", bufs=2, space="PSUM") as ps:
            w_t = cst.tile([feat, hidden], _DT)
            nc.sync.dma_start(out=w_t[:, :], in_=wd[:, :])
            b_t = cst.tile([hidden, 1], _DT)
            nc.sync.dma_start(out=b_t[:, :], in_=bd[:, :])
            id_t = cst.tile([P, P], _DT)
            nc.sync.dma_start(out=id_t[:, :], in_=ident[:, :])

            part = big.tile([P, ROWS_PP, feat], _DT)
            for st in range(n_stripes):
                mt = sb.tile([P, SS, feat], _DT, tag="mt")
                nc.sync.dma_start(out=mt[:, :, :],
                                  in_=msg[:, st * SS:(st + 1) * SS, :])
                tt = sb.tile([P, SS], _DT, tag="tt")
                nc.scalar.dma_start(out=tt[:, :],
                                    in_=tsd[:, st * SS:(st + 1) * SS])
                nc.vector.tensor_tensor(
                    out=mt[:, :, :], in0=mt[:, :, :],
                    in1=tt[:, :, None].to_broadcast([P, SS, feat]),
                    op=mybir.AluOpType.mult)
                nc.vector.tensor_reduce(
                    out=part[:, st * STRIPE_ROWS:(st + 1) * STRIPE_ROWS, :],
                    in_=mt[:, :, :].rearrange("p (rows s) f -> p rows f s", s=16),
                    axis=mybir.AxisListType.X, op=mybir.AluOpType.max)

            agg = big.tile([P, M, feat], _DT)
            row0 = node0 = 0
            for r, m in cfg["GROUPS"]:
                nc.vector.tensor_reduce(
                    out=agg[:, node0:node0 + m, :],
                    in_=part[:, row0:row0 + r * m, :].rearrange(
                        "p (m r) f -> p m f r", r=r),
                    axis=mybir.AxisListType.X, op=mybir.AluOpType.max)
                row0 += r * m
                node0 += m

            func = (mybir.ActivationFunctionType.Relu if relu
                    else mybir.ActivationFunctionType.Identity)
            for ms in range(M):
                atp = ps.tile([feat, P], _DT, tag="tp")
                nc.tensor.transpose(out=atp[:, :], in_=agg[:, ms, :],
                                    identity=id_t[:, :])
                ats = sb.tile([feat, P], _DT, tag="ats")
                nc.scalar.copy(out=ats[:, :], in_=atp[:, :])
                hp = ps.tile([hidden, P], _DT, tag="hp")
                nc.tensor.matmul(out=hp[:, :], lhsT=w_t[:, :], rhs=ats[:, :],
                                 start=True, stop=True)
                hs = sb.tile([hidden, P], _DT, tag="hs")
                nc.scalar.activation(out=hs[:, :], in_=hp[:, :], func=func,
                                     bias=b_t[:, :], scale=1.0)
                nc.sync.dma_start(out=hT[:, ms, :], in_=hs[:, :])
    _split_waits(nc)
    return nc


# ------------------------------------------------------------------- kernel
_CACHE = {}
LAST_TIMINGS = {}


def kernel(x, src, dst, timestamp, W1, b1, W2, b2):
    x = np.ascontiguousarray(np.asarray(x, np.float32))
    src = np.asarray(src, np.int32)
    dst = np.asarray(dst, np.int32)
    timestamp = np.asarray(timestamp, np.float32)
    W1 = np.asarray(W1, np.float32)
    b1 = np.asarray(b1, np.float32)
    W2 = np.asarray(W2, np.float32)
    b2 = np.asarray(b2, np.float32)

    shards, cfg = _prepare(src, dst, timestamp)
    M, S_PP = cfg["M"], cfg["S_PP"]
    identv = np.eye(P, dtype=np.float32)
    cores = list(range(N_CORES))

    # ---- launch 1: expand x (per-edge gather on device)
    import time as _time
    nc1 = _build_expand(cfg, F1)
    in1 = [{"xp": np.ascontiguousarray(x[sh.xrows])} for sh in shards]
    _t = _time.time()
    r1 = run_bass_kernel_spmd(nc1, in1, cores).results
    LAST_TIMINGS["expand_x"] = _time.time() - _t

    # ---- host: permute device-produced messages into dst slot tables
    in2 = []
    for n, sh in enumerate(shards):
        msg_flat = r1[n]["msgs"]                       # [ETOT, F1]
        pos = sh.pos_of_edge[sh.slot_edge]             # [P, S_PP]
        in2.append({
            "msg": np.ascontiguousarray(msg_flat[pos]),
            "ts": np.ascontiguousarray(sh.ts[sh.slot_edge]),
            "w": W1, "b": np.ascontiguousarray(b1[:, None]),
            "ident": identv,
        })

    # ---- launch 2: scale + segment max + linear1 + relu
    nc2 = _build_reduce(cfg, F1, HID, relu=True)
    _t = _time.time()
    r2 = run_bass_kernel_spmd(nc2, in2, cores).results
    LAST_TIMINGS["reduce_1"] = _time.time() - _t

    h_full = np.zeros((N_NODES, HID), np.float32)
    for n, sh in enumerate(shards):
        hT = r2[n]["hT"]                               # [HID, M, P]
        hb = hT.transpose(1, 2, 0)                     # [M, P, HID]
        valid = sh.node_grid >= 0
        h_full[n * B + sh.node_grid[valid]] = hb[valid]

    # ---- launch 3: expand h
    nc3 = _build_expand(cfg, HID)
    in3 = [{"xp": np.ascontiguousarray(h_full[sh.xrows])} for sh in shards]
    _t = _time.time()
    r3 = run_bass_kernel_spmd(nc3, in3, cores).results
    LAST_TIMINGS["expand_h"] = _time.time() - _t

    # ---- launch 4: scale + segment max + linear2
    in4 = []
    for n, sh in enumerate(shards):
        msg_flat = r3[n]["msgs"]
        pos = sh.pos_of_edge[sh.slot_edge]
        in4.append({
            "msg": np.ascontiguousarray(msg_flat[pos]),
            "ts": np.ascontiguousarray(sh.ts[sh.slot_edge]),
            "w": W2, "b": np.ascontiguousarray(b2[:, None]),
            "ident": identv,
        })
    nc4 = _build_reduce(cfg, HID, NCLS, relu=False)
    _t = _time.time()
    r4 = run_bass_kernel_spmd(nc4, in4, cores).results
    LAST_TIMINGS["reduce_2"] = _time.time() - _t

    out = np.zeros((N_NODES, NCLS), np.float32)
    for n, sh in enumerate(shards):
        oT = r4[n]["hT"]
        ob = oT.transpose(1, 2, 0)
        valid = sh.node_grid >= 0
        out[n * B + sh.node_grid[valid]] = ob[valid]
    return out



# revision 2
# speedup vs baseline: 14.7522x; 14.7522x over previous
"""GCN (gather-scale-segment_max x2) fused into ONE SPMD launch on 8 TRN2 cores.

Sharding: dst-node blocks across cores (each core owns the full reduction for
its 12500 nodes — no cross-core reduce); src nodes split into 8 "octants" of
12500 mapped to the 8 16-partition SBUF groups, and each octant into 2 halves
of 6250 so the fp16 feature-major gather tables stay under the gpsimd
indirect-copy data-size limit (~22KB/partition).

On device, per core:
  AllGather(x block, fp16) -> resident [128, BP] table (partition 16o+f holds
    feature f of octant o); per (half) chunk: gpsimd indirect_copy gathers
    per-edge features, PE mask-matmul broadcasts ts across the 16-partition
    groups, DVE multiplies (f32) and runs the length-classed segmented max
    into fp16 partials laid out in 8192-slot ranges (last slot of each range
    is a -60000 neutral).
  gather-2 jobs (round x range) collect per-(dst,octant,half) partials ->
    DMA octant shuffle -> cross-octant max -> W+b (+relu) -> h1 (fp16)
  AllGather(h1) -> L2 tables; same streams for layer 2 -> out (f32).

Host does only index/stream construction (outside the timed launch).
"""
import os

os.environ.setdefault("JAX_COMPILATION_CACHE_DIR", "/tmp/jax_kernel_cache")

import hashlib
import numpy as np
import ml_dtypes
from concourse import bass, mybir
from concourse.bass_utils import run_bass_kernel_spmd
from concourse.tile import TileContext
from bass_rust import ScopedClock

try:
    import jax
    jax.config.update("jax_compilation_cache_dir", "/tmp/jax_kernel_cache")
    jax.config.update("jax_persistent_cache_min_compile_time_secs", 0.5)
except Exception:
    pass

N_CORES = 8
N_NODES = 100000
B = N_NODES // N_CORES          # dst nodes per core
OCT = 8
BO = N_NODES // OCT             # src nodes per octant
HALF = 2
BH = BO // HALF                 # src nodes per half (6250)
F1, HID, NCLS = 16, 8, 2
KMAX = 32                       # max segment length
CH_CAP = 1024                   # stream positions per SBUF chunk
GB = 256                        # dst nodes per gather-2/combine chunk
BP = ((B + 15) // 16) * 16      # padded per-core dst count (12512)
P = 128
RANGE = 8192                    # partial positions per gather-2 range
NEUT = RANGE - 1                # local neutral slot within each range

_DT = mybir.dt.float32
_BF = mybir.dt.float16
_U16 = mybir.dt.uint16
BF_NP = np.float16


# ---------------------------------------------------------------- tile patch
class _Tc(TileContext):
    """This walrus build allows only ONE sync-wait per instruction; split the
    end-of-kernel drain waits across SP nops."""

    def _drain_and_barrier(self, tick_clock, wait_clock):
        holder = self.nc.sync.nop(nofuse=True, hint="drain_waits")
        wait_clock.add_sem_waits(holder.ins, ScopedClock({None: tick_clock.global_clock}))
        si = holder.ins.sync_info
        waits = list(si.on_wait) if si and si.on_wait else []
        if len(waits) > 1:
            upd = list(si.on_update) if si.on_update else []
            holder.ins.sync_info = mybir.SyncInfo(on_wait=waits[:1], on_update=upd)
            for w in waits[1:]:
                extra = self.nc.sync.nop(nofuse=True, hint="drain_waits")
                extra.ins.sync_info = mybir.SyncInfo(on_wait=[w], on_update=[])
        self.nc.sync.drain()
        self.nc.all_engine_barrier()
        assert self.sems is not None
        popped = self.nc._tile_sem_poison_stack.pop()
        assert popped is self._sem_poison
        self.nc.clear_and_free_semaphores(list(self.sems.allocated().values()))
        self.nc.all_engine_barrier()


def _split_waits(nc, max_waits=1):
    n = 0
    for fn in nc.m.functions:
        for bb in fn.blocks:
            out = []
            for inst in bb.instructions:
                si = inst.sync_info
                waits = list(si.on_wait) if si and si.on_wait else []
                if len(waits) > max_waits:
                    for w in waits[:-max_waits]:
                        n += 1
                        nop = mybir.InstNoOp(name=f"I-ws-{n}")
                        nop.engine = inst.engine
                        nop.sync_info = mybir.SyncInfo(on_wait=[w], on_update=[])
                        out.append(nop)
                    inst.sync_info = mybir.SyncInfo(
                        on_wait=waits[-max_waits:],
                        on_update=list(si.on_update) if si.on_update else [],
                    )
                out.append(inst)
            bb.instructions[:] = out
    return n


# ------------------------------------------------------------- host indexing
def _ragged_arange(counts):
    counts = np.asarray(counts, dtype=np.int64)
    total = int(counts.sum())
    cum = np.cumsum(counts) - counts
    return np.arange(total, dtype=np.int64) - np.repeat(cum, counts)


def _pos_phys(L):
    """Logical partial index -> physical position (skip slot NEUT per RANGE)."""
    return (L // NEUT) * RANGE + (L % NEUT)


def _prepare(src, dst, ts):
    src = np.asarray(src, np.int64)
    dst = np.asarray(dst, np.int64)
    ts = np.asarray(ts, np.float32)

    core = dst // B
    octant = src // BO
    half = (src % BO) // BH
    d_loc = dst - core * B
    s_loc = src % BO - half * BH            # < BH

    NCELL = N_CORES * OCT * HALF * B
    cell = ((core * OCT + octant) * HALF + half) * B + d_loc
    deg = np.bincount(cell, minlength=NCELL)
    nfull = deg // KMAX
    rem = deg % KMAX
    nseg_cell = nfull + (rem > 0)

    # class sizes per half: n_k[h][k] = max over (c,o) groups
    nk = np.zeros((N_CORES * OCT * HALF, KMAX + 1), np.int64)
    for g in range(N_CORES * OCT * HALF):
        sl = slice(g * B, (g + 1) * B)
        nk[g] = np.bincount(rem[sl][deg[sl] > 0], minlength=KMAX + 1)
        nk[g, KMAX] += nfull[sl].sum()
    nk[:, 0] = 0
    n_k = nk.reshape(N_CORES * OCT, HALF, KMAX + 1).max(axis=0)   # [HALF, KMAX+1]

    # global stream layout: halves sequential; chunks of whole segments
    chunks = []          # (h, length, runs); run = (off, nseg, k, Lppos0)
    cur_len = 0
    cur_runs = []
    Lctr = 0             # logical partial counter
    seg_off_k = {}
    seg_ppos_k = {}
    stream_off = 0
    cur_h = 0

    def _close_chunk():
        nonlocal cur_len, cur_runs, stream_off
        pad = (-cur_len) % 16
        cur_len += pad
        stream_off += pad
        chunks.append((cur_h, cur_len, cur_runs))
        cur_len = 0
        cur_runs = []

    for h in range(HALF):
        cur_h = h
        for k in range(1, KMAX + 1):
            if n_k[h, k] == 0:
                continue
            offs = np.empty(n_k[h, k], np.int64)
            lpos = np.empty(n_k[h, k], np.int64)
            left = int(n_k[h, k])
            slot = 0
            while left > 0:
                space = (CH_CAP - cur_len) // k
                if space <= 0:
                    _close_chunk()
                    continue
                take = min(space, left)
                offs[slot:slot + take] = stream_off + np.arange(take) * k
                lpos[slot:slot + take] = Lctr + np.arange(take)
                cur_runs.append((cur_len, int(take), k, Lctr))
                cur_len += take * k
                stream_off += take * k
                Lctr += take
                slot += take
                left -= take
            seg_off_k[(h, k)] = offs
            seg_ppos_k[(h, k)] = _pos_phys(lpos)
        if cur_len:
            _close_chunk()
    SP = stream_off
    NPART_L = Lctr
    n_ranges = (int(_pos_phys(NPART_L - 1)) // RANGE + 1) if NPART_L else 1
    NPALLOC = n_ranges * RANGE
    assert NPALLOC <= 65536 and SP % 16 == 0

    # convert runs' logical ppos to physical, splitting runs that straddle a
    # skipped neutral slot
    chunks2 = []
    for h, chlen, runs in chunks:
        rr = []
        for off, nseg, k, L0 in runs:
            start = 0
            while start < nseg:
                Ls = L0 + start
                room = NEUT - (Ls % NEUT)
                take = min(nseg - start, room)
                rr.append((off + start * k, int(take), k, int(_pos_phys(Ls))))
                start += take
        chunks2.append((h, chlen, rr))
    chunks = chunks2

    # per-(c,o,h): class-slot assignment for actual segments
    seg_base = np.concatenate([[0], np.cumsum(nseg_cell)])
    total_segs = int(seg_base[-1])
    seg_cell = np.repeat(np.arange(NCELL), nseg_cell)
    seg_i = _ragged_arange(nseg_cell)
    seg_k = np.where(seg_i < nfull[seg_cell], KMAX, rem[seg_cell])
    seg_grp = seg_cell // B                   # (c,o,h)

    sort_key = seg_grp * (KMAX + 1) + seg_k
    order = np.argsort(sort_key, kind="stable")
    slot_in_grp = _ragged_arange(
        np.bincount(sort_key[order], minlength=(N_CORES * OCT * HALF) * (KMAX + 1)))
    seg_stream = np.empty(total_segs, np.int64)
    seg_ppos = np.empty(total_segs, np.int64)
    ko = seg_k[order]
    ho = seg_grp[order] % HALF
    for h in range(HALF):
        for k in range(1, KMAX + 1):
            if (h, k) not in seg_off_k:
                continue
            m = (ko == k) & (ho == h)
            if m.any():
                seg_stream[order[m]] = seg_off_k[(h, k)][slot_in_grp[m]]
                seg_ppos[order[m]] = seg_ppos_k[(h, k)][slot_in_grp[m]]

    # edge -> stream position
    eorder = np.lexsort((d_loc, half, octant, core))
    ekey = cell[eorder]
    grp_counts = np.bincount(ekey, minlength=NCELL)
    r = _ragged_arange(grp_counts[np.unique(ekey)])
    eseg = seg_base[ekey] + r // KMAX
    epos = seg_stream[eseg] + r % KMAX

    idx_stream = np.zeros((N_CORES, OCT, SP), np.uint16)
    ts_stream = np.zeros((N_CORES, OCT, SP), np.float32)
    ec = core[eorder]
    eo = octant[eorder]
    idx_stream[ec, eo, epos] = s_loc[eorder].astype(np.uint16)
    ts_stream[ec, eo, epos] = ts[eorder]

    # gather-2 rounds: per (c,o,d) the segments of each half in order
    nseg4 = nseg_cell.reshape(N_CORES, OCT, HALF, B)
    cellbase4 = seg_base[:-1].reshape(N_CORES, OCT, HALF, B)
    rounds = []          # pos arrays [C, OCT, B], global neutral = NEUT
    max_h = nseg4.max(axis=(0, 1, 3))        # [HALF]
    for h in range(HALF):
        for i in range(int(max_h[h])):
            posr = np.full((N_CORES, OCT, B), NEUT, np.int64)
            m = nseg4[:, :, h, :] > i
            posr[m] = seg_ppos[cellbase4[:, :, h, :][m] + i]
            rounds.append(posr)

    # jobs: (round, range) pairs with real content
    jobs = []
    idx2_list = []
    for rd, posr in enumerate(rounds):
        for j in range(n_ranges):
            inr = (posr // RANGE == j) & (posr % RANGE != NEUT)
            if not inr.any():
                continue
            loc = np.where(inr, posr - j * RANGE, NEUT)
            full = np.full((N_CORES, OCT, BP), NEUT, np.uint16)
            full[:, :, :B] = loc.astype(np.uint16)
            jobs.append((rd, j))
            idx2_list.append(full)
    idx2 = np.stack(idx2_list)               # [n_jobs, C, OCT, BP]

    cfg = dict(SP=SP, NPALLOC=NPALLOC, n_ranges=n_ranges, chunks=chunks,
               n_jobs=len(jobs), jobs=jobs)
    return cfg, idx_stream, ts_stream, idx2


def _wrap16(stream):
    """[OCT, S] -> tile [128, S/16] with position j at (16o + j%16, j//16)."""
    o, s = stream.shape
    assert s % 16 == 0
    return stream.reshape(o, s // 16, 16).transpose(0, 2, 1).reshape(o * 16, s // 16)


# ------------------------------------------------------------ device build
def _build(cfg, loop_n=1, ablate=()):
    SP, NPALLOC, chunks = cfg["SP"], cfg["NPALLOC"], cfg["chunks"]
    n_jobs, jobs = cfg["n_jobs"], cfg["jobs"]

    nc = bass.Bass("TRN2", target_bir_lowering=False, debug=False,
                   num_devices=N_CORES)
    xt_d = nc.declare_dram_parameter("xt", [16, BP], _BF, isOutput=False)
    idx_d = nc.declare_dram_parameter("idxs", [P, SP // 16], _U16, isOutput=False)
    ts_d = nc.declare_dram_parameter("tss", [8, SP], _BF, isOutput=False)
    idx2_d = nc.declare_dram_parameter("idx2", [P, n_jobs, BP // 16], _U16, isOutput=False)
    mask_d = nc.declare_dram_parameter("mask", [8, P], _BF, isOutput=False)
    w1_d = nc.declare_dram_parameter("w1", [F1, HID], _BF, isOutput=False)
    b1_d = nc.declare_dram_parameter("b1", [HID, 1], _DT, isOutput=False)
    w2_d = nc.declare_dram_parameter("w2", [HID, NCLS], _BF, isOutput=False)
    b2_d = nc.declare_dram_parameter("b2", [NCLS, 1], _DT, isOutput=False)
    out_d = nc.declare_dram_parameter("outT", [NCLS, BP], _DT, isOutput=True)

    xt_int = nc.dram_tensor("xt_int", [16, BP], _BF, kind="Internal")
    xg_int = nc.dram_tensor("xg_int", [P, BP], _BF, kind="Internal", addr_space="Shared")
    h1_int = nc.dram_tensor("h1_int", [HID, BP], _BF, kind="Internal")
    hg_int = nc.dram_tensor("hg_int", [OCT * HID, BP], _BF, kind="Internal",
                            addr_space="Shared")
    rg = [list(range(N_CORES))]

    with _Tc(nc) as tc:
        with tc.tile_pool(name="cst", bufs=1) as cst, \
             tc.tile_pool(name="sb", bufs=3) as sb, \
             tc.tile_pool(name="g2p", bufs=2) as g2p, \
             tc.tile_pool(name="ps", bufs=2, space="PSUM") as ps, \
             tc.tile_pool(name="ps2", bufs=2, space="PSUM") as ps2:
            mask = cst.tile([8, P], _BF)
            nc.sync.dma_start(out=mask[:, :], in_=mask_d[:, :])
            w1 = cst.tile([F1, HID], _BF)
            nc.sync.dma_start(out=w1[:, :], in_=w1_d[:, :])
            b1 = cst.tile([HID, 1], _DT)
            nc.sync.dma_start(out=b1[:, :], in_=b1_d[:, :])
            w2 = cst.tile([HID, NCLS], _BF)
            nc.sync.dma_start(out=w2[:, :], in_=w2_d[:, :])
            b2 = cst.tile([NCLS, 1], _DT)
            nc.sync.dma_start(out=b2[:, :], in_=b2_d[:, :])
            idx2_t = cst.tile([P, n_jobs, BP // 16], _U16)
            nc.sync.dma_start(out=idx2_t[:, :, :], in_=idx2_d[:, :, :])
            table = cst.tile([P, BP], _BF)
            part = cst.tile([P, NPALLOC], _BF)

            from contextlib import ExitStack
            with ExitStack() as _st:
                # collectives cannot run inside a hardware loop: for the
                # repeat-measurement build (loop_n > 1) hoist them pre-loop
                # (their cost is measured separately by cc_meas.py)
                nc.sync.dma_start(out=xt_int[:, :], in_=xt_d[:, :])
                nc.gpsimd.collective_compute(
                    "AllGather", mybir.AluOpType.bypass, replica_groups=rg,
                    ins=[xt_int[:, :]], outs=[xg_int[:, :]])
                if loop_n > 1:
                    nc.gpsimd.collective_compute(
                        "AllGather", mybir.AluOpType.bypass, replica_groups=rg,
                        ins=[h1_int[:, :]], outs=[hg_int[:, :]])
                    _st.enter_context(tc.For_i(0, loop_n, 1))
                nc.sync.dma_start(out=table[:, :], in_=xg_int[:, :])

                for layer in range(2):
                    wt, bt = (w1, b1) if layer == 0 else (w2, b2)
                    fin = F1 if layer == 0 else HID
                    hid = HID if layer == 0 else NCLS
                    func = (mybir.ActivationFunctionType.Relu if layer == 0
                            else mybir.ActivationFunctionType.Identity)

                    nc.vector.memset(part[:, :], -60000.0)

                    # ---- stream phase
                    off = 0
                    for h, chlen, runs in chunks:
                        c16 = chlen // 16
                        o16 = off // 16
                        idxc = sb.tile([P, CH_CAP // 16], _U16, tag="idx")
                        nc.sync.dma_start(out=idxc[:, :c16],
                                          in_=idx_d[:, o16:o16 + c16])
                        tsc = sb.tile([8, CH_CAP], _BF, tag="ts")
                        nc.scalar.dma_start(out=tsc[:, :chlen],
                                            in_=ts_d[:, off:off + chlen])
                        msgb = sb.tile([P, CH_CAP, 1], _BF, tag="msgb")
                        if "sgather" in ablate:
                            nc.gpsimd.memset(msgb[:, :1, :], 0)
                        else:
                            nc.gpsimd.indirect_copy(
                                out=msgb[:, :chlen, :],
                                data=table[:, h * BH:(h + 1) * BH],
                                idxs=idxc[:, :c16], i_know_ap_gather_is_preferred=True)
                        msgf = sb.tile([P, CH_CAP], _DT, tag="msgf")
                        if "mult" in ablate:
                            nc.vector.memset(msgf[:, :1], 0)
                        else:
                            for s in range(0, chlen, 512):
                                w = min(512, chlen - s)
                                tst = ps.tile([P, 512], _DT, tag="tst")
                                nc.tensor.matmul(out=tst[:, :w], lhsT=mask[:, :],
                                                 rhs=tsc[:, s:s + w], start=True, stop=True)
                                nc.vector.tensor_tensor(
                                    out=msgf[:, s:s + w], in0=msgb[:, s:s + w, 0],
                                    in1=tst[:, :w], op=mybir.AluOpType.mult)
                        if "reduce" not in ablate:
                            for (roff, nseg, k, ppos0) in runs:
                                nc.vector.tensor_reduce(
                                    out=part[:, ppos0:ppos0 + nseg],
                                    in_=msgf[:, roff:roff + nseg * k].rearrange(
                                        "p (n k) -> p n k", k=k),
                                    axis=mybir.AxisListType.X, op=mybir.AluOpType.max)
                        off += chlen

                    # ---- combine phase
                    for gb in range(0, BP, GB):
                        gw = min(GB, BP - gb)
                        g16 = gw // 16
                        gb16 = gb // 16
                        g2 = g2p.tile([P, GB, 1], _BF, tag="g2")
                        g2b = g2p.tile([P, GB, 1], _BF, tag="g2b")
                        for ji, (rd, rj) in enumerate(jobs):
                            dst_t = g2 if ji == 0 else g2b
                            if "g2gather" in ablate:
                                nc.gpsimd.memset(dst_t[:, :1, :], 0)
                            else:
                                nc.gpsimd.indirect_copy(
                                    out=dst_t[:, :gw, :],
                                    data=part[:, rj * RANGE:(rj + 1) * RANGE],
                                    idxs=idx2_t[:, ji, gb16:gb16 + g16],
                                    i_know_ap_gather_is_preferred=True)
                            if ji > 0:
                                nc.vector.tensor_tensor(
                                    out=g2[:, :gw, 0], in0=g2[:, :gw, 0],
                                    in1=g2b[:, :gw, 0], op=mybir.AluOpType.max)
                        gr = g2p.tile([16, OCT, GB], _BF, tag="gr")
                        for o in range(OCT):
                            nc.sync.dma_start(out=gr[:, o, :gw],
                                              in_=g2[16 * o:16 * (o + 1), :gw, 0])
                        agg = g2p.tile([16, GB], _BF, tag="agg")
                        nc.vector.tensor_reduce(
                            out=agg[:, :gw],
                            in_=gr[:, :, :gw].rearrange("p o b -> p b o"),
                            axis=mybir.AxisListType.X, op=mybir.AluOpType.max)
                        hp = ps2.tile([HID, GB], _DT, tag="hp")
                        nc.tensor.matmul(out=hp[:hid, :gw], lhsT=wt[:, :],
                                         rhs=agg[:fin, :gw], start=True, stop=True)
                        if layer == 0:
                            hs = g2p.tile([HID, GB], _BF, tag="hs")
                            nc.scalar.activation(out=hs[:hid, :gw], in_=hp[:hid, :gw],
                                                 func=func, bias=bt[:, :], scale=1.0)
                            nc.scalar.dma_start(out=h1_int[:, gb:gb + gw],
                                                in_=hs[:hid, :gw])
                        else:
                            hs = g2p.tile([HID, GB], _DT, tag="hsf")
                            nc.scalar.activation(out=hs[:hid, :gw], in_=hp[:hid, :gw],
                                                 func=func, bias=bt[:, :], scale=1.0)
                            nc.scalar.dma_start(out=out_d[:, gb:gb + gw],
                                                in_=hs[:hid, :gw])

                    if layer == 0:
                        if loop_n == 1:
                            nc.gpsimd.collective_compute(
                                "AllGather", mybir.AluOpType.bypass, replica_groups=rg,
                                ins=[h1_int[:, :]], outs=[hg_int[:, :]])
                        nc.vector.memset(table[:, :], 0.0)
                        for o in range(OCT):
                            nc.sync.dma_start(
                                out=table[16 * o:16 * o + HID, :],
                                in_=hg_int[HID * o:HID * (o + 1), :])
    _split_waits(nc)
    return nc


def _build_loop(cfg, loop_n):
    return _build(cfg, loop_n=loop_n)


# ------------------------------------------------------------------- kernel
_CACHE = {}
LAST_TIMINGS = {}


def _fingerprint(*arrs):
    h = hashlib.sha1()
    for a in arrs:
        h.update(np.ascontiguousarray(a).tobytes())
    return h.hexdigest()


def _get_plan(src, dst, timestamp):
    fp = _fingerprint(src, dst, timestamp)
    if fp not in _CACHE:
        cfg, idx_stream, ts_stream, idx2 = _prepare(src, dst, timestamp)
        idx_tiles = np.stack([_wrap16(idx_stream[c]) for c in range(N_CORES)])
        idx2_tiles = np.stack(
            [np.stack([_wrap16(idx2[j, c]) for j in range(cfg["n_jobs"])], axis=1)
             for c in range(N_CORES)])  # [C, 128, n_jobs, BP/16]
        mask = np.zeros((8, P), BF_NP)
        for o in range(OCT):
            mask[o, 16 * o:16 * (o + 1)] = 1.0
        nc = _build(cfg)
        _CACHE[fp] = (cfg, idx_tiles, ts_stream, idx2_tiles, mask, nc)
    return _CACHE[fp]


def _in_maps(x, W1, b1, W2, b2, idx_tiles, ts_stream, idx2_tiles, mask):
    xtb = np.zeros((N_CORES, 16, BP), BF_NP)
    for c in range(N_CORES):
        xtb[c, :, :B] = x[c * BO:(c + 1) * BO].T.astype(BF_NP)
    maps = []
    for c in range(N_CORES):
        maps.append({
            "xt": xtb[c],
            "idxs": idx_tiles[c],
            "tss": np.ascontiguousarray(ts_stream[c]).astype(BF_NP),
            "idx2": idx2_tiles[c],
            "mask": mask,
            "w1": np.asarray(W1, np.float32).astype(BF_NP),
            "b1": np.ascontiguousarray(np.asarray(b1, np.float32)[:, None]),
            "w2": np.asarray(W2, np.float32).astype(BF_NP),
            "b2": np.ascontiguousarray(np.asarray(b2, np.float32)[:, None]),
        })
    return maps


def kernel(x, src, dst, timestamp, W1, b1, W2, b2):
    x = np.ascontiguousarray(np.asarray(x, np.float32))
    src = np.asarray(src, np.int32)
    dst = np.asarray(dst, np.int32)
    timestamp = np.asarray(timestamp, np.float32)

    cfg, idx_tiles, ts_stream, idx2_tiles, mask, nc = _get_plan(src, dst, timestamp)
    maps = _in_maps(x, W1, b1, W2, b2, idx_tiles, ts_stream, idx2_tiles, mask)

    import time as _time
    t0 = _time.time()
    res = run_bass_kernel_spmd(nc, maps, list(range(N_CORES))).results
    LAST_TIMINGS["fused"] = _time.time() - t0

    out = np.empty((N_NODES, NCLS), np.float32)
    for c in range(N_CORES):
        out[c * B:(c + 1) * B] = res[c]["outT"][:, :B].T
    return out


# revision 3
# speedup vs baseline: 17.6324x; 1.1952x over previous
"""GCN (gather-scale-segment_max x2) fused into ONE SPMD launch on 8 TRN2 cores.

Sharding: dst-node blocks across cores (each core owns the full reduction for
its 12500 nodes — no cross-core reduce); src nodes split into 8 "octants" of
12500 mapped to the 8 16-partition SBUF groups, and each octant into 2 halves
of 6250 so the fp16 feature-major gather tables stay under the gpsimd
indirect-copy data-size limit (~22KB/partition).

On device, per core:
  AllGather(x block, fp16) -> resident [128, BP] table (partition 16o+f holds
    feature f of octant o); per (half) chunk: gpsimd indirect_copy gathers
    per-edge features, PE mask-matmul broadcasts ts across the 16-partition
    groups, DVE multiplies (f32) and runs the length-classed segmented max
    into fp16 partials laid out in 8192-slot ranges (last slot of each range
    is a -60000 neutral).
  gather-2 jobs (round x range) collect per-(dst,octant,half) partials ->
    DMA octant shuffle -> cross-octant max -> W+b (+relu) -> h1 (fp16)
  AllGather(h1) -> L2 tables; same streams for layer 2 -> out (f32).

Host does only index/stream construction (outside the timed launch).
"""
import os

os.environ.setdefault("JAX_COMPILATION_CACHE_DIR", "/tmp/jax_kernel_cache")

import hashlib
import numpy as np
import ml_dtypes
from concourse import bass, mybir
from concourse.bass_utils import run_bass_kernel_spmd
from concourse.tile import TileContext
from bass_rust import ScopedClock

try:
    import jax
    jax.config.update("jax_compilation_cache_dir", "/tmp/jax_kernel_cache")
    jax.config.update("jax_persistent_cache_min_compile_time_secs", 0.5)
except Exception:
    pass

N_CORES = 8
N_NODES = 100000
B = N_NODES // N_CORES          # dst nodes per core
OCT = 8
BO = N_NODES // OCT             # src nodes per octant
HALF = 2
BH = BO // HALF                 # src nodes per half (6250)
F1, HID, NCLS = 16, 8, 2
KMAX = 32                       # max segment length
CH_CAP = 1024                   # stream positions per SBUF chunk
GB = 256                        # dst nodes per gather-2/combine chunk
BP = ((B + 15) // 16) * 16      # padded per-core dst count (12512)
P = 128
RANGE = 11264                   # partial positions per gather-2 range (22528B fp16 slice)
NEUT = RANGE - 1                # local neutral slot within each range

_DT = mybir.dt.float32
_BF = mybir.dt.float16
_U16 = mybir.dt.uint16
BF_NP = np.float16


# ---------------------------------------------------------------- tile patch
class _Tc(TileContext):
    """This walrus build allows only ONE sync-wait per instruction; split the
    end-of-kernel drain waits across SP nops."""

    def _drain_and_barrier(self, tick_clock, wait_clock):
        holder = self.nc.sync.nop(nofuse=True, hint="drain_waits")
        wait_clock.add_sem_waits(holder.ins, ScopedClock({None: tick_clock.global_clock}))
        si = holder.ins.sync_info
        waits = list(si.on_wait) if si and si.on_wait else []
        if len(waits) > 1:
            upd = list(si.on_update) if si.on_update else []
            holder.ins.sync_info = mybir.SyncInfo(on_wait=waits[:1], on_update=upd)
            for w in waits[1:]:
                extra = self.nc.sync.nop(nofuse=True, hint="drain_waits")
                extra.ins.sync_info = mybir.SyncInfo(on_wait=[w], on_update=[])
        self.nc.sync.drain()
        self.nc.all_engine_barrier()
        assert self.sems is not None
        popped = self.nc._tile_sem_poison_stack.pop()
        assert popped is self._sem_poison
        self.nc.clear_and_free_semaphores(list(self.sems.allocated().values()))
        self.nc.all_engine_barrier()


def _split_waits(nc, max_waits=1):
    n = 0
    for fn in nc.m.functions:
        for bb in fn.blocks:
            out = []
            for inst in bb.instructions:
                si = inst.sync_info
                waits = list(si.on_wait) if si and si.on_wait else []
                if len(waits) > max_waits:
                    for w in waits[:-max_waits]:
                        n += 1
                        nop = mybir.InstNoOp(name=f"I-ws-{n}")
                        nop.engine = inst.engine
                        nop.sync_info = mybir.SyncInfo(on_wait=[w], on_update=[])
                        out.append(nop)
                    inst.sync_info = mybir.SyncInfo(
                        on_wait=waits[-max_waits:],
                        on_update=list(si.on_update) if si.on_update else [],
                    )
                out.append(inst)
            bb.instructions[:] = out
    return n


# ------------------------------------------------------------- host indexing
def _ragged_arange(counts):
    counts = np.asarray(counts, dtype=np.int64)
    total = int(counts.sum())
    cum = np.cumsum(counts) - counts
    return np.arange(total, dtype=np.int64) - np.repeat(cum, counts)


def _pos_phys(L):
    """Logical partial index -> physical position (skip slot NEUT per RANGE)."""
    return (L // NEUT) * RANGE + (L % NEUT)


def _prepare(src, dst, ts):
    src = np.asarray(src, np.int64)
    dst = np.asarray(dst, np.int64)
    ts = np.asarray(ts, np.float32)

    core = dst // B
    octant = src // BO
    half = (src % BO) // BH
    d_loc = dst - core * B
    s_loc = src % BO - half * BH            # < BH

    NCELL = N_CORES * OCT * HALF * B
    cell = ((core * OCT + octant) * HALF + half) * B + d_loc
    deg = np.bincount(cell, minlength=NCELL)
    nfull = deg // KMAX
    rem = deg % KMAX
    nseg_cell = nfull + (rem > 0)

    # class sizes per half: n_k[h][k] = max over (c,o) groups
    nk = np.zeros((N_CORES * OCT * HALF, KMAX + 1), np.int64)
    for g in range(N_CORES * OCT * HALF):
        sl = slice(g * B, (g + 1) * B)
        nk[g] = np.bincount(rem[sl][deg[sl] > 0], minlength=KMAX + 1)
        nk[g, KMAX] += nfull[sl].sum()
    nk[:, 0] = 0
    n_k = nk.reshape(N_CORES * OCT, HALF, KMAX + 1).max(axis=0)   # [HALF, KMAX+1]

    # global stream layout: halves sequential; chunks of whole segments
    chunks = []          # (h, length, runs); run = (off, nseg, k, Lppos0)
    cur_len = 0
    cur_runs = []
    Lctr = 0             # logical partial counter
    seg_off_k = {}
    seg_ppos_k = {}
    stream_off = 0
    cur_h = 0

    def _close_chunk():
        nonlocal cur_len, cur_runs, stream_off
        pad = (-cur_len) % 16
        cur_len += pad
        stream_off += pad
        chunks.append((cur_h, cur_len, cur_runs))
        cur_len = 0
        cur_runs = []

    for h in range(HALF):
        cur_h = h
        for k in range(1, KMAX + 1):
            if n_k[h, k] == 0:
                continue
            offs = np.empty(n_k[h, k], np.int64)
            lpos = np.empty(n_k[h, k], np.int64)
            left = int(n_k[h, k])
            slot = 0
            while left > 0:
                space = (CH_CAP - cur_len) // k
                if space <= 0:
                    _close_chunk()
                    continue
                take = min(space, left)
                offs[slot:slot + take] = stream_off + np.arange(take) * k
                lpos[slot:slot + take] = Lctr + np.arange(take)
                cur_runs.append((cur_len, int(take), k, Lctr))
                cur_len += take * k
                stream_off += take * k
                Lctr += take
                slot += take
                left -= take
            seg_off_k[(h, k)] = offs
            seg_ppos_k[(h, k)] = _pos_phys(lpos)
        if cur_len:
            _close_chunk()
    SP = stream_off
    NPART_L = Lctr
    n_ranges = (int(_pos_phys(NPART_L - 1)) // RANGE + 1) if NPART_L else 1
    NPALLOC = n_ranges * RANGE
    assert NPALLOC <= 65536 and SP % 16 == 0

    # convert runs' logical ppos to physical, splitting runs that straddle a
    # skipped neutral slot
    chunks2 = []
    for h, chlen, runs in chunks:
        rr = []
        for off, nseg, k, L0 in runs:
            start = 0
            while start < nseg:
                Ls = L0 + start
                room = NEUT - (Ls % NEUT)
                take = min(nseg - start, room)
                rr.append((off + start * k, int(take), k, int(_pos_phys(Ls))))
                start += take
        chunks2.append((h, chlen, rr))
    chunks = chunks2

    # per-(c,o,h): class-slot assignment for actual segments
    seg_base = np.concatenate([[0], np.cumsum(nseg_cell)])
    total_segs = int(seg_base[-1])
    seg_cell = np.repeat(np.arange(NCELL), nseg_cell)
    seg_i = _ragged_arange(nseg_cell)
    seg_k = np.where(seg_i < nfull[seg_cell], KMAX, rem[seg_cell])
    seg_grp = seg_cell // B                   # (c,o,h)

    sort_key = seg_grp * (KMAX + 1) + seg_k
    order = np.argsort(sort_key, kind="stable")
    slot_in_grp = _ragged_arange(
        np.bincount(sort_key[order], minlength=(N_CORES * OCT * HALF) * (KMAX + 1)))
    seg_stream = np.empty(total_segs, np.int64)
    seg_ppos = np.empty(total_segs, np.int64)
    ko = seg_k[order]
    ho = seg_grp[order] % HALF
    for h in range(HALF):
        for k in range(1, KMAX + 1):
            if (h, k) not in seg_off_k:
                continue
            m = (ko == k) & (ho == h)
            if m.any():
                seg_stream[order[m]] = seg_off_k[(h, k)][slot_in_grp[m]]
                seg_ppos[order[m]] = seg_ppos_k[(h, k)][slot_in_grp[m]]

    # edge -> stream position
    eorder = np.lexsort((d_loc, half, octant, core))
    ekey = cell[eorder]
    grp_counts = np.bincount(ekey, minlength=NCELL)
    r = _ragged_arange(grp_counts[np.unique(ekey)])
    eseg = seg_base[ekey] + r // KMAX
    epos = seg_stream[eseg] + r % KMAX

    idx_stream = np.zeros((N_CORES, OCT, SP), np.uint16)
    ts_stream = np.zeros((N_CORES, OCT, SP), np.float32)
    ec = core[eorder]
    eo = octant[eorder]
    idx_stream[ec, eo, epos] = s_loc[eorder].astype(np.uint16)
    ts_stream[ec, eo, epos] = ts[eorder]

    # gather-2 rounds: per (c,o,d) the segments of each half in order
    nseg4 = nseg_cell.reshape(N_CORES, OCT, HALF, B)
    cellbase4 = seg_base[:-1].reshape(N_CORES, OCT, HALF, B)
    rounds = []          # pos arrays [C, OCT, B], global neutral = NEUT
    max_h = nseg4.max(axis=(0, 1, 3))        # [HALF]
    for h in range(HALF):
        for i in range(int(max_h[h])):
            posr = np.full((N_CORES, OCT, B), NEUT, np.int64)
            m = nseg4[:, :, h, :] > i
            posr[m] = seg_ppos[cellbase4[:, :, h, :][m] + i]
            rounds.append(posr)

    # jobs: (round, range) pairs with real content
    jobs = []
    idx2_list = []
    for rd, posr in enumerate(rounds):
        for j in range(n_ranges):
            inr = (posr // RANGE == j) & (posr % RANGE != NEUT)
            if not inr.any():
                continue
            loc = np.where(inr, posr - j * RANGE, NEUT)
            full = np.full((N_CORES, OCT, BP), NEUT, np.uint16)
            full[:, :, :B] = loc.astype(np.uint16)
            jobs.append((rd, j))
            idx2_list.append(full)
    idx2 = np.stack(idx2_list)               # [n_jobs, C, OCT, BP]

    cfg = dict(SP=SP, NPALLOC=NPALLOC, n_ranges=n_ranges, chunks=chunks,
               n_jobs=len(jobs), jobs=jobs)
    return cfg, idx_stream, ts_stream, idx2


def _wrap16(stream):
    """[OCT, S] -> tile [128, S/16] with position j at (16o + j%16, j//16)."""
    o, s = stream.shape
    assert s % 16 == 0
    return stream.reshape(o, s // 16, 16).transpose(0, 2, 1).reshape(o * 16, s // 16)


# ------------------------------------------------------------ device build
def _build(cfg, loop_n=1, ablate=()):
    SP, NPALLOC, chunks = cfg["SP"], cfg["NPALLOC"], cfg["chunks"]
    n_jobs, jobs = cfg["n_jobs"], cfg["jobs"]

    nc = bass.Bass("TRN2", target_bir_lowering=False, debug=False,
                   num_devices=N_CORES)
    xt_d = nc.declare_dram_parameter("xt", [16, BP], _BF, isOutput=False)
    idx_d = nc.declare_dram_parameter("idxs", [P, SP // 16], _U16, isOutput=False)
    ts_d = nc.declare_dram_parameter("tss", [8, SP], _BF, isOutput=False)
    idx2_d = nc.declare_dram_parameter("idx2", [P, n_jobs, BP // 16], _U16, isOutput=False)
    mask_d = nc.declare_dram_parameter("mask", [8, P], _BF, isOutput=False)
    w1_d = nc.declare_dram_parameter("w1", [F1, HID], _BF, isOutput=False)
    b1_d = nc.declare_dram_parameter("b1", [HID, 1], _DT, isOutput=False)
    w2_d = nc.declare_dram_parameter("w2", [HID, NCLS], _BF, isOutput=False)
    b2_d = nc.declare_dram_parameter("b2", [NCLS, 1], _DT, isOutput=False)
    out_d = nc.declare_dram_parameter("outT", [NCLS, BP], _DT, isOutput=True)

    xt_int = nc.dram_tensor("xt_int", [16, BP], _BF, kind="Internal")
    xg_int = nc.dram_tensor("xg_int", [P, BP], _BF, kind="Internal", addr_space="Shared")
    h1_int = nc.dram_tensor("h1_int", [HID, BP], _BF, kind="Internal")
    hg_int = nc.dram_tensor("hg_int", [OCT * HID, BP], _BF, kind="Internal",
                            addr_space="Shared")
    rg = [list(range(N_CORES))]

    with _Tc(nc) as tc:
        with tc.tile_pool(name="cst", bufs=1) as cst, \
             tc.tile_pool(name="sb", bufs=3) as sb, \
             tc.tile_pool(name="g2p", bufs=2) as g2p, \
             tc.tile_pool(name="ps", bufs=2, space="PSUM") as ps, \
             tc.tile_pool(name="ps2", bufs=2, space="PSUM") as ps2:
            mask = cst.tile([8, P], _BF)
            nc.sync.dma_start(out=mask[:, :], in_=mask_d[:, :])
            w1 = cst.tile([F1, HID], _BF)
            nc.sync.dma_start(out=w1[:, :], in_=w1_d[:, :])
            b1 = cst.tile([HID, 1], _DT)
            nc.sync.dma_start(out=b1[:, :], in_=b1_d[:, :])
            w2 = cst.tile([HID, NCLS], _BF)
            nc.sync.dma_start(out=w2[:, :], in_=w2_d[:, :])
            b2 = cst.tile([NCLS, 1], _DT)
            nc.sync.dma_start(out=b2[:, :], in_=b2_d[:, :])
            idx2_t = cst.tile([P, n_jobs, BP // 16], _U16)
            nc.sync.dma_start(out=idx2_t[:, :, :], in_=idx2_d[:, :, :])
            table = cst.tile([P, BP], _BF)
            part = cst.tile([P, NPALLOC], _BF)

            from contextlib import ExitStack
            with ExitStack() as _st:
                # collectives cannot run inside a hardware loop: for the
                # repeat-measurement build (loop_n > 1) hoist them pre-loop
                # (their cost is measured separately by cc_meas.py)
                nc.sync.dma_start(out=xt_int[:, :], in_=xt_d[:, :])
                nc.gpsimd.collective_compute(
                    "AllGather", mybir.AluOpType.bypass, replica_groups=rg,
                    ins=[xt_int[:, :]], outs=[xg_int[:, :]])
                if loop_n > 1:
                    nc.gpsimd.collective_compute(
                        "AllGather", mybir.AluOpType.bypass, replica_groups=rg,
                        ins=[h1_int[:, :]], outs=[hg_int[:, :]])
                    _st.enter_context(tc.For_i(0, loop_n, 1))
                nc.sync.dma_start(out=table[:, :], in_=xg_int[:, :])

                for layer in range(2):
                    wt, bt = (w1, b1) if layer == 0 else (w2, b2)
                    fin = F1 if layer == 0 else HID
                    hid = HID if layer == 0 else NCLS
                    func = (mybir.ActivationFunctionType.Relu if layer == 0
                            else mybir.ActivationFunctionType.Identity)

                    nc.vector.memset(part[:, :], -60000.0)

                    # ---- stream phase
                    off = 0
                    for h, chlen, runs in chunks:
                        c16 = chlen // 16
                        o16 = off // 16
                        idxc = sb.tile([P, CH_CAP // 16], _U16, tag="idx")
                        nc.sync.dma_start(out=idxc[:, :c16],
                                          in_=idx_d[:, o16:o16 + c16])
                        tsc = sb.tile([8, CH_CAP], _BF, tag="ts")
                        nc.scalar.dma_start(out=tsc[:, :chlen],
                                            in_=ts_d[:, off:off + chlen])
                        msgb = sb.tile([P, CH_CAP, 1], _BF, tag="msgb")
                        if "sgather" in ablate:
                            nc.gpsimd.memset(msgb[:, :1, :], 0)
                        else:
                            nc.gpsimd.indirect_copy(
                                out=msgb[:, :chlen, :],
                                data=table[:, h * BH:(h + 1) * BH],
                                idxs=idxc[:, :c16], i_know_ap_gather_is_preferred=True)
                        msgf = sb.tile([P, CH_CAP], _DT, tag="msgf")
                        if "mult" in ablate:
                            nc.vector.memset(msgf[:, :1], 0)
                        else:
                            for s in range(0, chlen, 512):
                                w = min(512, chlen - s)
                                tst = ps.tile([P, 512], _DT, tag="tst")
                                nc.tensor.matmul(out=tst[:, :w], lhsT=mask[:, :],
                                                 rhs=tsc[:, s:s + w], start=True, stop=True)
                                nc.vector.tensor_tensor(
                                    out=msgf[:, s:s + w], in0=msgb[:, s:s + w, 0],
                                    in1=tst[:, :w], op=mybir.AluOpType.mult)
                        if "reduce" not in ablate:
                            for (roff, nseg, k, ppos0) in runs:
                                nc.vector.tensor_reduce(
                                    out=part[:, ppos0:ppos0 + nseg],
                                    in_=msgf[:, roff:roff + nseg * k].rearrange(
                                        "p (n k) -> p n k", k=k),
                                    axis=mybir.AxisListType.X, op=mybir.AluOpType.max)
                        off += chlen

                    # ---- combine phase
                    for gb in range(0, BP, GB):
                        gw = min(GB, BP - gb)
                        g16 = gw // 16
                        gb16 = gb // 16
                        g2 = g2p.tile([P, GB, 1], _BF, tag="g2")
                        g2b = g2p.tile([P, GB, 1], _BF, tag="g2b")
                        for ji, (rd, rj) in enumerate(jobs):
                            dst_t = g2 if ji == 0 else g2b
                            if "g2gather" in ablate:
                                nc.gpsimd.memset(dst_t[:, :1, :], 0)
                            else:
                                nc.gpsimd.indirect_copy(
                                    out=dst_t[:, :gw, :],
                                    data=part[:, rj * RANGE:(rj + 1) * RANGE],
                                    idxs=idx2_t[:, ji, gb16:gb16 + g16],
                                    i_know_ap_gather_is_preferred=True)
                            if ji > 0:
                                nc.vector.tensor_tensor(
                                    out=g2[:, :gw, 0], in0=g2[:, :gw, 0],
                                    in1=g2b[:, :gw, 0], op=mybir.AluOpType.max)
                        gr = g2p.tile([16, OCT, GB], _BF, tag="gr")
                        for o in range(OCT):
                            nc.sync.dma_start(out=gr[:, o, :gw],
                                              in_=g2[16 * o:16 * (o + 1), :gw, 0])
                        agg = g2p.tile([16, GB], _BF, tag="agg")
                        nc.vector.tensor_reduce(
                            out=agg[:, :gw],
                            in_=gr[:, :, :gw].rearrange("p o b -> p b o"),
                            axis=mybir.AxisListType.X, op=mybir.AluOpType.max)
                        hp = ps2.tile([HID, GB], _DT, tag="hp")
                        nc.tensor.matmul(out=hp[:hid, :gw], lhsT=wt[:, :],
                                         rhs=agg[:fin, :gw], start=True, stop=True)
                        if layer == 0:
                            hs = g2p.tile([HID, GB], _BF, tag="hs")
                            nc.scalar.activation(out=hs[:hid, :gw], in_=hp[:hid, :gw],
                                                 func=func, bias=bt[:, :], scale=1.0)
                            nc.scalar.dma_start(out=h1_int[:, gb:gb + gw],
                                                in_=hs[:hid, :gw])
                        else:
                            hs = g2p.tile([HID, GB], _DT, tag="hsf")
                            nc.scalar.activation(out=hs[:hid, :gw], in_=hp[:hid, :gw],
                                                 func=func, bias=bt[:, :], scale=1.0)
                            nc.scalar.dma_start(out=out_d[:, gb:gb + gw],
                                                in_=hs[:hid, :gw])

                    if layer == 0:
                        if loop_n == 1:
                            nc.gpsimd.collective_compute(
                                "AllGather", mybir.AluOpType.bypass, replica_groups=rg,
                                ins=[h1_int[:, :]], outs=[hg_int[:, :]])
                        nc.vector.memset(table[:, :], 0.0)
                        for o in range(OCT):
                            nc.sync.dma_start(
                                out=table[16 * o:16 * o + HID, :],
                                in_=hg_int[HID * o:HID * (o + 1), :])
    _split_waits(nc)
    return nc


def _build_loop(cfg, loop_n):
    return _build(cfg, loop_n=loop_n)


# ------------------------------------------------------------------- kernel
_CACHE = {}
LAST_TIMINGS = {}


def _fingerprint(*arrs):
    h = hashlib.sha1()
    for a in arrs:
        h.update(np.ascontiguousarray(a).tobytes())
    return h.hexdigest()


def _get_plan(src, dst, timestamp):
    fp = _fingerprint(src, dst, timestamp)
    if fp not in _CACHE:
        cfg, idx_stream, ts_stream, idx2 = _prepare(src, dst, timestamp)
        idx_tiles = np.stack([_wrap16(idx_stream[c]) for c in range(N_CORES)])
        idx2_tiles = np.stack(
            [np.stack([_wrap16(idx2[j, c]) for j in range(cfg["n_jobs"])], axis=1)
             for c in range(N_CORES)])  # [C, 128, n_jobs, BP/16]
        mask = np.zeros((8, P), BF_NP)
        for o in range(OCT):
            mask[o, 16 * o:16 * (o + 1)] = 1.0
        nc = _build(cfg)
        _CACHE[fp] = (cfg, idx_tiles, ts_stream, idx2_tiles, mask, nc)
    return _CACHE[fp]


def _in_maps(x, W1, b1, W2, b2, idx_tiles, ts_stream, idx2_tiles, mask):
    xtb = np.zeros((N_CORES, 16, BP), BF_NP)
    for c in range(N_CORES):
        xtb[c, :, :B] = x[c * BO:(c + 1) * BO].T.astype(BF_NP)
    maps = []
    for c in range(N_CORES):
        maps.append({
            "xt": xtb[c],
            "idxs": idx_tiles[c],
            "tss": np.ascontiguousarray(ts_stream[c]).astype(BF_NP),
            "idx2": idx2_tiles[c],
            "mask": mask,
            "w1": np.asarray(W1, np.float32).astype(BF_NP),
            "b1": np.ascontiguousarray(np.asarray(b1, np.float32)[:, None]),
            "w2": np.asarray(W2, np.float32).astype(BF_NP),
            "b2": np.ascontiguousarray(np.asarray(b2, np.float32)[:, None]),
        })
    return maps


def kernel(x, src, dst, timestamp, W1, b1, W2, b2):
    x = np.ascontiguousarray(np.asarray(x, np.float32))
    src = np.asarray(src, np.int32)
    dst = np.asarray(dst, np.int32)
    timestamp = np.asarray(timestamp, np.float32)

    cfg, idx_tiles, ts_stream, idx2_tiles, mask, nc = _get_plan(src, dst, timestamp)
    maps = _in_maps(x, W1, b1, W2, b2, idx_tiles, ts_stream, idx2_tiles, mask)

    import time as _time
    t0 = _time.time()
    res = run_bass_kernel_spmd(nc, maps, list(range(N_CORES))).results
    LAST_TIMINGS["fused"] = _time.time() - t0

    out = np.empty((N_NODES, NCLS), np.float32)
    for c in range(N_CORES):
        out[c * B:(c + 1) * B] = res[c]["outT"][:, :B].T
    return out
